# revision 4
# baseline (speedup 1.0000x reference)
"""BCH/RS systematic encoder kernel for Trainium2 (8 NeuronCores, data parallel).

Computes out = concat([msg, (msg @ Gp) mod 2], axis=-1) for
msg [16384, 1000] f32 of 0/1 bits and Gp [1000, 256] f32 of 0/1 bits.

Design (per core, 2048 rows, 16 chunks of 128):
  - SWDGE cast-load msg chunk f32 -> bf16 SBUF (0/1 exact in bf16)
  - SWDGE cast-store bf16 -> f32 to out[:, :1000] (systematic copy-through)
  - DMA xbar transpose (2-byte) 128x128 blocks: msg natural -> msgT [k, m]
  - 8 accumulating bf16 matmuls: psum[m,256] += msgT_k.T @ Gp_k (fp32 accum, exact)
  - DVE tensor_scalar mod 2.0 on psum -> SBUF f32
  - store parity to out[:, 1000:1256]
HBM traffic/core = 8.19 MB read + 10.29 MB write (the minimum).
"""

import os
import sys

import numpy as np

if os.path.isdir("/opt/trn_rl_repo") and "/opt/trn_rl_repo" not in sys.path:
    sys.path.insert(0, "/opt/trn_rl_repo")

import ml_dtypes

import concourse.bacc as bacc
import concourse.mybir as mybir
import concourse.tile as tile
from concourse.bass_utils import run_bass_kernel_spmd

BATCH = 16384
MSG = 1000
NPAR = 256
NCORES = 8
ROWS = BATCH // NCORES  # 2048
P = 128
KCH = 8  # k chunks; padded K = 1024
KPAD = KCH * P

# test.py pokes these for profiling
TRACE = False
LAST_RESULT = None

_CACHE = {}


def build_nc(rows=ROWS):
    """Emit the Bass/Tile IR for one core handling `rows` rows."""
    mch = rows // P
    nc = bacc.Bacc("TRN2", target_bir_lowering=False, debug=False)
    msg = nc.dram_tensor("msg", [rows, MSG], mybir.dt.float32, kind="ExternalInput")
    gp = nc.dram_tensor("gp", [P, KCH * NPAR], mybir.dt.bfloat16, kind="ExternalInput")
    out = nc.dram_tensor(
        "out", [rows, MSG + NPAR], mybir.dt.float32, kind="ExternalOutput"
    )

    with tile.TileContext(nc) as tc:
        with (
            tc.tile_pool(name="gpool", bufs=1) as gpool,
            tc.tile_pool(name="apool", bufs=4) as apool,
            tc.tile_pool(name="bpool", bufs=3) as bpool,
            tc.tile_pool(name="cpool", bufs=4) as cpool,
            tc.tile_pool(name="dpool", bufs=4) as dpool,
            tc.tile_pool(name="ppool", bufs=4, space="PSUM") as ppool,
        ):
            # Gp resident in SBUF: gsb[q, kb*256 + n] = Gp_padded[kb*128 + q, n]
            gsb = gpool.tile([P, KCH * NPAR], mybir.dt.bfloat16)
            nc.sync.dma_start(out=gsb[:, :], in_=gp[:, :])

            for ci in range(mch):
                r0 = ci * P
                # bf16 msg chunk, natural layout [m, k], zero-padded to k=1024
                a = apool.tile([P, KPAD], mybir.dt.bfloat16, tag="a")
                nc.gpsimd.dma_start(out=a[:, 0:MSG], in_=msg[r0 : r0 + P, :])
                nc.vector.memset(a[:, MSG:KPAD], 0)
                # systematic copy-through (bf16 -> f32 widening is exact)
                nc.gpsimd.dma_start(out=out[r0 : r0 + P, 0:MSG], in_=a[:, 0:MSG])

                # xbar transpose each 128x128 block: b[q, kb*128+p] = a[p, kb*128+q]
                b = bpool.tile([P, KPAD], mybir.dt.bfloat16, tag="b")
                for kb in range(KCH):
                    nc.sync.dma_start(
                        out=b[:, kb * P : (kb + 1) * P],
                        in_=a[:, kb * P : (kb + 1) * P],
                        transpose=True,
                    )

                acc = ppool.tile([P, NPAR], mybir.dt.float32, tag="acc")
                for kb in range(KCH):
                    nc.tensor.matmul(
                        acc[:, :],
                        b[:, kb * P : (kb + 1) * P],
                        gsb[:, kb * NPAR : (kb + 1) * NPAR],
                        start=(kb == 0),
                        stop=(kb == KCH - 1),
                    )

                # parity mod 2 on exact integers: ACT f32->i32, DVE AND 1 (i32),
                # DVE i32->f32 (bitVec ops cannot cast, so convert separately)
                c = cpool.tile([P, NPAR], mybir.dt.int32, tag="c")
                nc.scalar.copy(c[:, :], acc[:, :])
                e = cpool.tile([P, NPAR], mybir.dt.int32, tag="e")
                nc.vector.tensor_scalar(
                    e[:, :], c[:, :], 1, None, mybir.AluOpType.bitwise_and
                )
                d = dpool.tile([P, NPAR], mybir.dt.float32, tag="d")
                nc.vector.tensor_copy(d[:, :], e[:, :])
                nc.sync.dma_start(out=out[r0 : r0 + P, MSG : MSG + NPAR], in_=d[:, :])

    nc.compile()
    return nc


def prep_gp(Gp):
    """Pad Gp to 1024 rows and swizzle to the [128, 8*256] bf16 SBUF layout."""
    gp = np.asarray(Gp, dtype=np.float32)
    gp_pad = np.zeros((KPAD, NPAR), dtype=np.float32)
    gp_pad[:MSG] = gp
    gsw = gp_pad.reshape(KCH, P, NPAR).transpose(1, 0, 2).reshape(P, KCH * NPAR)
    return np.ascontiguousarray(gsw).astype(ml_dtypes.bfloat16)


def kernel(message_bits, Gp):
    global LAST_RESULT
    msg = np.ascontiguousarray(np.asarray(message_bits, dtype=np.float32))
    assert msg.shape == (BATCH, MSG), msg.shape
    gsw = prep_gp(Gp)

    if "nc" not in _CACHE:
        _CACHE["nc"] = build_nc()
    nc = _CACHE["nc"]

    in_maps = [
        {"msg": msg[i * ROWS : (i + 1) * ROWS], "gp": gsw} for i in range(NCORES)
    ]
    res = run_bass_kernel_spmd(
        nc, in_maps, core_ids=list(range(NCORES)), trace=TRACE
    )
    LAST_RESULT = res
    return np.concatenate([r["out"] for r in res.results], axis=0)


# revision 5
# speedup vs baseline: 2.4323x; 2.4323x over previous
"""BCH/RS systematic encoder kernel for Trainium2 (8 NeuronCores, data parallel).

Computes out = concat([msg, (msg @ Gp) mod 2], axis=-1) for
msg [16384, 1000] f32 of 0/1 bits and Gp [1000, 256] f32 of 0/1 bits.

Design (per core, 2048 rows, 16 chunks of 128):
  - SWDGE cast-load msg chunk f32 -> bf16 SBUF (0/1 exact in bf16)
  - SWDGE cast-store bf16 -> f32 to out[:, :1000] (systematic copy-through)
  - DMA xbar transpose (2-byte) 128x128 blocks: msg natural -> msgT [k, m]
  - 8 accumulating bf16 matmuls: psum[m,256] += msgT_k.T @ Gp_k (fp32 accum, exact)
  - DVE tensor_scalar mod 2.0 on psum -> SBUF f32
  - store parity to out[:, 1000:1256]
HBM traffic/core = 8.19 MB read + 10.29 MB write (the minimum).
"""

import os
import sys

import numpy as np

if os.path.isdir("/opt/trn_rl_repo") and "/opt/trn_rl_repo" not in sys.path:
    sys.path.insert(0, "/opt/trn_rl_repo")

import ml_dtypes

import concourse.bacc as bacc
import concourse.mybir as mybir
import concourse.tile as tile
from concourse.bass_utils import run_bass_kernel_spmd

BATCH = 16384
MSG = 1000
NPAR = 256
NCORES = 8
ROWS = BATCH // NCORES  # 2048
P = 128
KCH = 8  # k chunks; padded K = 1024
KPAD = KCH * P

# test.py pokes these for profiling
TRACE = False
LAST_RESULT = None

_CACHE = {}


def build_nc(rows=ROWS):
    """Emit the Bass/Tile IR for one core handling `rows` rows."""
    mch = rows // P
    nc = bacc.Bacc("TRN2", target_bir_lowering=False, debug=False)
    msg = nc.dram_tensor("msg", [rows, MSG], mybir.dt.float32, kind="ExternalInput")
    gp = nc.dram_tensor("gp", [P, KCH * NPAR], mybir.dt.bfloat16, kind="ExternalInput")
    out = nc.dram_tensor(
        "out", [rows, MSG + NPAR], mybir.dt.float32, kind="ExternalOutput"
    )

    with tile.TileContext(nc) as tc:
        with (
            tc.tile_pool(name="gpool", bufs=1) as gpool,
            tc.tile_pool(name="apool", bufs=4) as apool,
            tc.tile_pool(name="bpool", bufs=3) as bpool,
            tc.tile_pool(name="cpool", bufs=4) as cpool,
            tc.tile_pool(name="dpool", bufs=4) as dpool,
            tc.tile_pool(name="ppool", bufs=4, space="PSUM") as ppool,
        ):
            # Gp resident in SBUF: gsb[q, kb*256 + n] = Gp_padded[kb*128 + q, n]
            gsb = gpool.tile([P, KCH * NPAR], mybir.dt.bfloat16)
            nc.sync.dma_start(out=gsb[:, :], in_=gp[:, :])

            for ci in range(mch):
                r0 = ci * P
                # bf16 msg chunk, natural layout [m, k], zero-padded to k=1024
                a = apool.tile([P, KPAD], mybir.dt.bfloat16, tag="a")
                nc.gpsimd.dma_start(out=a[:, 0:MSG], in_=msg[r0 : r0 + P, :])
                nc.vector.memset(a[:, MSG:KPAD], 0)
                # systematic copy-through (bf16 -> f32 widening is exact)
                nc.gpsimd.dma_start(out=out[r0 : r0 + P, 0:MSG], in_=a[:, 0:MSG])

                # one xbar transpose for the whole chunk (3D out, b-major):
                # b[q, kb, p] = a[p, kb*128 + q], alternate the two HWDGE engines
                b = bpool.tile([P, KCH, P], mybir.dt.bfloat16, tag="b")
                hwdge = nc.sync if ci % 2 == 0 else nc.scalar
                hwdge.dma_start(out=b[:, :, :], in_=a[:, :], transpose=True)

                acc = ppool.tile([P, NPAR], mybir.dt.float32, tag="acc")
                for kb in range(KCH):
                    nc.tensor.matmul(
                        acc[:, :],
                        b[:, kb, :],
                        gsb[:, kb * NPAR : (kb + 1) * NPAR],
                        start=(kb == 0),
                        stop=(kb == KCH - 1),
                    )

                # parity mod 2 on exact integers: ACT f32->i32, DVE AND 1 (i32),
                # DVE i32->f32 (bitVec ops cannot cast, so convert separately)
                c = cpool.tile([P, NPAR], mybir.dt.int32, tag="c")
                nc.scalar.copy(c[:, :], acc[:, :])
                e = cpool.tile([P, NPAR], mybir.dt.int32, tag="e")
                nc.vector.tensor_scalar(
                    e[:, :], c[:, :], 1, None, mybir.AluOpType.bitwise_and
                )
                d = dpool.tile([P, NPAR], mybir.dt.float32, tag="d")
                nc.vector.tensor_copy(d[:, :], e[:, :])
                nc.sync.dma_start(out=out[r0 : r0 + P, MSG : MSG + NPAR], in_=d[:, :])

    nc.compile()
    return nc


def prep_gp(Gp):
    """Pad Gp to 1024 rows and swizzle to the [128, 8*256] bf16 SBUF layout."""
    gp = np.asarray(Gp, dtype=np.float32)
    gp_pad = np.zeros((KPAD, NPAR), dtype=np.float32)
    gp_pad[:MSG] = gp
    gsw = gp_pad.reshape(KCH, P, NPAR).transpose(1, 0, 2).reshape(P, KCH * NPAR)
    return np.ascontiguousarray(gsw).astype(ml_dtypes.bfloat16)


def kernel(message_bits, Gp):
    global LAST_RESULT
    msg = np.ascontiguousarray(np.asarray(message_bits, dtype=np.float32))
    assert msg.shape == (BATCH, MSG), msg.shape
    gsw = prep_gp(Gp)

    if "nc" not in _CACHE:
        _CACHE["nc"] = build_nc()
    nc = _CACHE["nc"]

    in_maps = [
        {"msg": msg[i * ROWS : (i + 1) * ROWS], "gp": gsw} for i in range(NCORES)
    ]
    res = run_bass_kernel_spmd(
        nc, in_maps, core_ids=list(range(NCORES)), trace=TRACE
    )
    LAST_RESULT = res
    return np.concatenate([r["out"] for r in res.results], axis=0)
